# revision 15
# baseline (speedup 1.0000x reference)
"""Tensor-parallel MultiHeadAttention (LN + fused QKV + causal SDPA + proj)
for 8 Trainium2 NeuronCores.

Sharding: 2 heads per core. LayerNorm gamma/beta folded into qkv weights on
host; LN (x-mu)*rstd applied via rank-1 PSUM corrections + evacuation scaling.
Heavy matmuls run in fp32r. Causal softmax computed on transposed scores
(scoresT[t,s]) so the softmax denominator is a PE ones-matmul.

Engine/queue layout (the perf-critical part):
  - SP queue (nc.sync):   all loads (xT, weights, const rows).
  - Act queue (nc.scalar): activations + rstd/rden broadcast DMAs + proj
    partial stores. Emission order makes every trigger's wait already
    satisfied, so the queue never head-blocks compute.
  - GpSimd queue:          ReduceScatter collectives ONLY.
  Nothing ever waits on a collective except the collective's own output,
  which IS the external output tensor (bf16); the host casts to f32.

Output layout per core (out_d [256, HID] bf16):
  sb in 0..3: rows [sb*64,(sb+1)*64) = global s rows sb*512 + core*64 ..
"""

import sys

sys.path.insert(0, "/opt/trn_rl_repo")

import math

import numpy as np

S, HID, NH, HD = 2048, 2048, 16, 128
EPS = 1e-5
NCORES = 8
HPC = NH // NCORES        # heads per core: 2
OQK = 2 * HPC * HD        # q+k rows per core: 512
OV = HPC * HD             # v rows per core: 256
KO = HID // 128           # contraction chunks: 16
NSB = S // 512            # s-blocks: 4
NTB = S // 128            # t-blocks: 16
RS_OUT = 512 // NCORES    # rows per core per full-sb RS chunk: 64
SCALE = 1.0 / math.sqrt(HD)
MASKVAL = -30000.0

_CACHE = {}


def _build_nc():
    import concourse.mybir as mybir
    import concourse.tile as tile
    from concourse import bacc
    from contextlib import ExitStack

    f32 = mybir.dt.float32
    f32r = mybir.dt.float32r
    bf16 = mybir.dt.bfloat16
    Act = mybir.ActivationFunctionType

    nc = bacc.Bacc(num_devices=NCORES)

    # ---- I/O ----
    xT_d = nc.dram_tensor("xT", [HID, S], f32r, kind="ExternalInput")
    wqkT_d = nc.dram_tensor("wqkT", [HID, OQK], f32r, kind="ExternalInput")
    wvT_d = nc.dram_tensor("wvT", [HID, OV], f32r, kind="ExternalInput")
    wpT_d = nc.dram_tensor("wpT", [OV, HID], f32r, kind="ExternalInput")
    rsb_qk_d = nc.dram_tensor("rsb_qk", [2, OQK], f32r, kind="ExternalInput")
    rsb_v_d = nc.dram_tensor("rsb_v", [2, OV], f32r, kind="ExternalInput")
    pbias8_d = nc.dram_tensor("pbias8", [1, HID], f32, kind="ExternalInput")
    maskneg_d = nc.dram_tensor("maskneg", [128, 128], bf16, kind="ExternalInput")
    ident_d = nc.dram_tensor("ident", [128, 128], bf16, kind="ExternalInput")
    ones_d = nc.dram_tensor("ones_col", [128, 1], f32r, kind="ExternalInput")
    out_d = nc.dram_tensor("out", [NSB * RS_OUT, HID], bf16, kind="ExternalOutput")

    # proj partial sums (bf16): whole-sb tensors for sb 0..2, per-st chunks
    # for the last sb so its ReduceScatter pipelines with its proj.
    part_dram = [
        nc.dram_tensor(f"part{sb}", [512, HID], bf16) for sb in range(NSB)
    ]
    rs_dram = [
        nc.dram_tensor(f"rs{sb}", [RS_OUT, HID], bf16) for sb in range(NSB)
    ]

    ctx = ExitStack()
    with ctx:
        tc = ctx.enter_context(tile.TileContext(nc))
        wpool = ctx.enter_context(tc.tile_pool(name="wpool", bufs=1))
        rows = ctx.enter_context(tc.tile_pool(name="rows", bufs=1))
        bigout = ctx.enter_context(tc.tile_pool(name="bigout", bufs=1))
        statrow = ctx.enter_context(tc.tile_pool(name="statrow", bufs=1))

        # resident small tiles (loads deferred into the sb0 h-loop so xt/w
        # hit the load queue first)
        ones_col = rows.tile([128, 1], f32r)
        nc.sync.dma_start(out=ones_col, in_=ones_d[:, :])
        eps_tile = rows.tile([128, 1], f32)
        nc.vector.memset(eps_tile, EPS)
        one11 = rows.tile([1, 1], f32)
        nc.vector.memset(one11, 1.0)
        rsb_qk = rows.tile([2, OQK], f32r)
        rsb_v = rows.tile([2, OV], f32r)
        maskneg = rows.tile([128, 128], bf16)
        ident = rows.tile([128, 128], bf16)
        pbias8_b = rows.tile([128, HID], f32)
        wpT = wpool.tile([128, HPC, HID], f32r)

        # persistent phase-1 outputs
        qkT = [bigout.tile([128, S], f32r, name=f"qkT{ob}") for ob in range(4)]
        vtile = bigout.tile([128, NTB, OV], f32r, name="vtile")
        ctxT = [bigout.tile([128, S], f32r, name=f"ctxT{h}") for h in range(HPC)]

        with (
            tc.tile_pool(name="wqkv", bufs=1) as wqkv,
            tc.tile_pool(name="xpool", bufs=6) as xpool,
            tc.tile_pool(name="sqpool", bufs=3) as sqpool,
            tc.tile_pool(name="rowr", bufs=2) as rowr,
            tc.tile_pool(name="bcast", bufs=2) as bcastp,
            tc.tile_pool(name="exppool", bufs=4) as exppool,
            tc.tile_pool(name="projpool", bufs=3) as projpool,
            tc.tile_pool(name="ps", bufs=8, space="PSUM") as psp,
        ):
            wqkT = wqkv.tile([128, KO, OQK], f32r)
            wvT = wqkv.tile([128, KO, OV], f32r)

            for sb in range(NSB):
                s0 = sb * 512
                # ---------------- phase 1: stats + qkT + v ----------------
                ps_stats = psp.tile([1, 512], f32, tag="bank", name="ps_stats")
                ps_sumsq = psp.tile([1, 512], f32, tag="bank", name="ps_sumsq")
                ps_qk = [
                    psp.tile([128, 512], f32, tag="bank", name=f"ps_qk{ob}")
                    for ob in range(4)
                ]
                ps_v = [
                    psp.tile([128, 512], f32, tag="bank", name=f"ps_v{i}")
                    for i in range(2)
                ]
                xsqs = []
                for h in range(KO):
                    xt = xpool.tile([128, 512], f32r, tag="xt", name=f"xt{sb}_{h}")
                    nc.sync.dma_start(
                        out=xt, in_=xT_d[h * 128 : (h + 1) * 128, s0 : s0 + 512]
                    )
                    if sb == 0:
                        nc.sync.dma_start(
                            out=wqkT[:, h, :], in_=wqkT_d[h * 128 : (h + 1) * 128, :]
                        )
                        nc.sync.dma_start(
                            out=wvT[:, h, :], in_=wvT_d[h * 128 : (h + 1) * 128, :]
                        )
                        if h == 2:
                            nc.sync.dma_start(out=rsb_qk, in_=rsb_qk_d[:, :])
                            nc.sync.dma_start(out=rsb_v, in_=rsb_v_d[:, :])
                        if h == 4:
                            nc.sync.dma_start(out=maskneg, in_=maskneg_d[:, :])
                            nc.sync.dma_start(out=ident, in_=ident_d[:, :])
                    xsq = sqpool.tile([128, 512], f32r, tag="xsq")
                    if h % 2 == 1:
                        nc.scalar.activation(out=xsq, in_=xt, func=Act.Square)
                    else:
                        nc.vector.tensor_mul(out=xsq, in0=xt, in1=xt)
                    xsqs.append(xsq)
                    nc.tensor.matmul(
                        ps_stats, ones_col, xt,
                        start=(h == 0), stop=(h == KO - 1),
                        skip_group_check=True,
                    )
                    for ob in range(4):
                        nc.tensor.matmul(
                            ps_qk[ob],
                            wqkT[:, h, ob * 128 : (ob + 1) * 128],
                            xt,
                            start=(h == 0),
                            stop=False,
                        )
                    for vs in range(4):
                        nc.tensor.matmul(
                            ps_v[vs // 2][:, (vs % 2) * 256 : (vs % 2 + 1) * 256],
                            xt[:, vs * 128 : (vs + 1) * 128],
                            wvT[:, h, :],
                            start=(h == 0 and vs % 2 == 0),
                            stop=False,
                            skip_group_check=(vs % 2 == 1),
                        )
                    # sumsq one chunk behind so the tensor queue never
                    # head-blocks on xsq availability
                    if h > 0:
                        nc.tensor.matmul(
                            ps_sumsq, ones_col, xsqs[h - 1],
                            start=(h == 1), stop=False,
                            skip_group_check=True,
                        )
                nc.tensor.matmul(
                    ps_sumsq, ones_col, xsqs[KO - 1],
                    start=False, stop=True,
                    skip_group_check=True,
                )
                xsqs = None
                if sb == 0:
                    nc.sync.dma_start(
                        out=pbias8_b, in_=pbias8_d[:, :].to_broadcast([128, HID])
                    )
                    nc.sync.dma_start(
                        out=wpT, in_=wpT_d.rearrange("(c p) o -> p c o", p=128)
                    )

                # stats rows
                stats2 = rowr.tile([2, 512], f32r, tag="stats2")
                negmu_r = rowr.tile([1, 512], f32r, tag="negmu_r")
                invrstd_r = rowr.tile([1, 512], f32r, tag="invrstd_r")
                nc.vector.tensor_scalar_mul(
                    out=negmu_r, in0=ps_stats, scalar1=-1.0 / HID
                )
                nc.scalar.dma_start(out=stats2[0:1, :], in_=negmu_r)
                mu2 = statrow.tile([1, 512], f32, tag="mu2")
                nc.vector.tensor_mul(out=mu2, in0=negmu_r, in1=negmu_r)
                var = statrow.tile([1, 512], f32, tag="var")
                nc.vector.scalar_tensor_tensor(
                    out=var,
                    in0=ps_sumsq,
                    scalar=1.0 / HID,
                    in1=mu2,
                    op0=mybir.AluOpType.mult,
                    op1=mybir.AluOpType.subtract,
                )
                nc.scalar.activation(
                    out=invrstd_r, in_=var, func=Act.Sqrt, bias=eps_tile[0:1]
                )
                nc.scalar.dma_start(out=stats2[1:2, :], in_=invrstd_r)
                rstd = statrow.tile([1, 512], f32, tag="rstd")
                nc.vector.reciprocal(out=rstd, in_=invrstd_r)
                rstd_b = bcastp.tile([128, 512], f32, tag="rstd_b")
                nc.gpsimd.partition_broadcast(rstd_b, rstd)

                # qk rank-1 corrections (negmu first, invrstd later) + column
                # rstd via PE transpose
                var_col_ps = psp.tile([128, 4], f32, tag="bank", name="var_col")
                for c in range(4):
                    nc.tensor.transpose(
                        var_col_ps[:, c : c + 1],
                        var[0:1, c * 128 : (c + 1) * 128],
                        one11,
                    )
                invrstd_col = statrow.tile([128, 4], f32, tag="invrstd_col")
                nc.scalar.activation(
                    out=invrstd_col, in_=var_col_ps, func=Act.Sqrt, bias=eps_tile
                )
                rstd_col = statrow.tile([128, 4], f32, tag="rstd_col")
                nc.vector.reciprocal(out=rstd_col, in_=invrstd_col)
                for ob in range(4):
                    nc.tensor.matmul(
                        ps_qk[ob],
                        rsb_qk[0:2, ob * 128 : (ob + 1) * 128],
                        stats2,
                        start=False,
                        stop=True,
                    )
                    nc.vector.tensor_mul(
                        out=qkT[ob][:, s0 : s0 + 512], in0=ps_qk[ob], in1=rstd_b
                    )
                for vs in range(4):
                    pv = ps_v[vs // 2][:, (vs % 2) * 256 : (vs % 2 + 1) * 256]
                    nc.tensor.matmul(
                        pv,
                        stats2[0:2, vs * 128 : (vs + 1) * 128],
                        rsb_v,
                        start=False,
                        stop=True,
                        skip_group_check=True,
                    )
                    nc.vector.tensor_scalar_mul(
                        out=vtile[:, sb * 4 + vs, :],
                        in0=pv,
                        scalar1=rstd_col[:, vs : vs + 1],
                    )

                # ---------------- attention for this sb ----------------
                ntb = 4 * (sb + 1)  # causal t-blocks
                for h in range(HPC):
                    qT = qkT[h]
                    kT = qkT[2 + h]
                    ps_ctx = psp.tile([128, 512], f32, tag="bank", name=f"ps_ctx{sb}_{h}")
                    ps_den = psp.tile([1, 512], f32, tag="bank", name=f"ps_den{sb}_{h}")
                    for tb in range(ntb):
                        t0 = tb * 128
                        delta = max(0, t0 - s0)
                        ps_sc = psp.tile([128, 512], f32, tag="bank", name="ps_sc")
                        nc.tensor.matmul(
                            ps_sc[:, delta:512],
                            kT[:, t0 : t0 + 128],
                            qT[:, s0 + delta : s0 + 512],
                            start=True,
                            stop=(t0 < s0),
                        )
                        if t0 >= s0:
                            nc.tensor.matmul(
                                ps_sc[:, delta : delta + 128],
                                maskneg,
                                ident,
                                start=False,
                                stop=True,
                            )
                        expt = exppool.tile([128, 512], f32r, tag="expt")
                        nc.scalar.activation(
                            out=expt[:, delta:512],
                            in_=ps_sc[:, delta:512],
                            func=Act.Exp,
                            scale=SCALE,
                        )
                        # columns [0, delta) are invalid (t > s) and never
                        # written: every column's first accumulant is tb==0.
                        nc.tensor.matmul(
                            ps_ctx[:, delta:512],
                            vtile[:, tb, h * HD : (h + 1) * HD],
                            expt[:, delta:512],
                            start=(tb == 0),
                            stop=(tb == ntb - 1),
                            skip_group_check=True,
                        )
                        nc.tensor.matmul(
                            ps_den[:, delta:512],
                            ones_col,
                            expt[:, delta:512],
                            start=(tb == 0),
                            stop=(tb == ntb - 1),
                            skip_group_check=True,
                        )
                    rden = statrow.tile([1, 512], f32, tag="rden")
                    nc.vector.reciprocal(out=rden, in_=ps_den)
                    rden_b = bcastp.tile([128, 512], f32, tag="rden_b")
                    nc.gpsimd.partition_broadcast(rden_b, rden)
                    nc.vector.tensor_mul(
                        out=ctxT[h][:, s0 : s0 + 512], in0=ps_ctx, in1=rden_b
                    )

                # ---------------- proj + reduce-scatter ----------------
                for st_i in range(4):
                    for ob in range(4):
                        o0 = ob * 512
                        ps_pr = psp.tile([128, 512], f32, tag="bank", name="ps_pr")
                        for h in range(HPC):
                            nc.tensor.matmul(
                                ps_pr,
                                ctxT[h][:, s0 + st_i * 128 : s0 + st_i * 128 + 128],
                                wpT[:, h, o0 : o0 + 512],
                                start=(h == 0),
                                stop=(h == HPC - 1),
                            )
                        ptile = projpool.tile([128, 512], bf16, tag="ptile")
                        nc.vector.tensor_add(
                            out=ptile, in0=ps_pr, in1=pbias8_b[:, o0 : o0 + 512]
                        )
                        nc.scalar.dma_start(
                            out=part_dram[sb][
                                st_i * 128 : (st_i + 1) * 128, o0 : o0 + 512
                            ],
                            in_=ptile,
                        )
                # previous sb's out copy goes on gpsimd just before this
                # sb's collective: its wait (prev collective) is satisfied
                # ~an sb-cycle ago, and only CC(sb) — whose inputs are not
                # ready earlier anyway — queues behind it. The final sb's
                # copy goes on the Act HW queue, where its dependency cone
                # covers every remaining Act instruction so the scheduler
                # cannot hoist it in front of compute.
                if sb >= 1:
                    nc.gpsimd.dma_start(
                        out=out_d[(sb - 1) * RS_OUT : sb * RS_OUT, :],
                        in_=rs_dram[sb - 1][:, :],
                    )
                nc.gpsimd.collective_compute(
                    "ReduceScatter",
                    mybir.AluOpType.add,
                    replica_groups=[list(range(NCORES))],
                    ins=[part_dram[sb].ap()],
                    outs=[rs_dram[sb].ap()],
                )
            nc.scalar.dma_start(
                out=out_d[3 * RS_OUT : 4 * RS_OUT, :],
                in_=rs_dram[3][:, :],
            )

    nc.finalize()
    return nc


def get_nc(debug=False, sim_mode=False):
    key = ("nc",)
    if key not in _CACHE:
        _CACHE[key] = _build_nc()
    return _CACHE[key]


def make_in_maps(hidden_states, ln_weight, ln_bias, qkv_weight, qkv_bias,
                 proj_weight, proj_bias):
    import ml_dtypes

    f4 = np.float32
    x = np.asarray(hidden_states, f4)[:, 0, :]                      # [S, HID]
    xT = np.ascontiguousarray(x.T)                                  # [HID, S]
    g = np.asarray(ln_weight, f4)
    b = np.asarray(ln_bias, f4)
    W = np.asarray(qkv_weight, f4)
    W1 = W * g[None, :]
    b1 = np.asarray(qkv_bias, f4) + W @ b
    W3 = W1.reshape(3, NH, HD, HID)
    b3 = b1.reshape(3, NH, HD)
    pw = np.asarray(proj_weight, f4)
    pb8 = (np.asarray(proj_bias, f4) / NCORES).reshape(1, HID)
    maskneg = np.triu(np.full((128, 128), MASKVAL, f4), 1).astype(ml_dtypes.bfloat16)
    ident = np.eye(128, dtype=ml_dtypes.bfloat16)
    ones_col = np.ones((128, 1), f4)

    in_maps = []
    for c in range(NCORES):
        hs = slice(HPC * c, HPC * (c + 1))
        Wq = W3[0, hs].reshape(OV, HID)
        Wk = W3[1, hs].reshape(OV, HID)
        Wv = W3[2, hs].reshape(OV, HID)
        Wqk = np.concatenate([Wq, Wk], 0)                           # [512, HID]
        in_maps.append({
            "xT": xT,
            "wqkT": np.ascontiguousarray(Wqk.T),
            "wvT": np.ascontiguousarray(Wv.T),
            "wpT": np.ascontiguousarray(pw[:, OV * c : OV * (c + 1)].T),
            "rsb_qk": np.stack([
                Wqk.sum(1),
                np.concatenate([b3[0, hs].reshape(OV), b3[1, hs].reshape(OV)]),
            ]),
            "rsb_v": np.stack([Wv.sum(1), b3[2, hs].reshape(OV)]),
            "pbias8": pb8,
            "maskneg": maskneg,
            "ident": ident,
            "ones_col": ones_col,
        })
    return in_maps


def assemble(outs):
    """outs: list of per-core [NSB*RS_OUT, HID] bf16 arrays -> [S, 1, HID] f32."""
    full = np.empty((S, HID), np.float32)
    for c in range(NCORES):
        o = np.asarray(outs[c], np.float32)
        for sb in range(NSB):
            full[sb * 512 + c * RS_OUT : sb * 512 + (c + 1) * RS_OUT, :] = o[
                sb * RS_OUT : (sb + 1) * RS_OUT, :
            ]
    return full.reshape(S, 1, HID)


class _Runner:
    """Cached PJRT runner: jit once, keep per-core weight shards device-
    resident across calls (re-uploaded only when weight bytes change)."""

    WEIGHT_NAMES = frozenset({
        "wqkT", "wvT", "wpT", "rsb_qk", "rsb_v", "pbias8",
        "maskneg", "ident", "ones_col",
    })

    def __init__(self, nc):
        import jax
        import concourse.mybir as mybir
        from concourse import bass2jax
        from concourse.bass2jax import _bass_exec_p, partition_id_tensor
        from jax.sharding import Mesh, PartitionSpec
        from jax.experimental.shard_map import shard_map

        bass2jax.install_neuronx_cc_hook()
        self.nc = nc
        self.jax = jax
        partition_name = (
            nc.partition_id_tensor.name if nc.partition_id_tensor else None
        )
        in_names, out_names, out_avals = [], [], []
        for alloc in nc.m.functions[0].allocations:
            if not isinstance(alloc, mybir.MemoryLocationSet):
                continue
            name = alloc.memorylocations[0].name
            if alloc.kind == "ExternalInput":
                if name != partition_name:
                    in_names.append(name)
            elif alloc.kind == "ExternalOutput":
                shape = tuple(alloc.tensor_shape)
                out_names.append(name)
                out_avals.append(
                    jax.core.ShapedArray(shape, mybir.dt.np(alloc.dtype))
                )
        self.in_names, self.out_names, self.out_avals = in_names, out_names, out_avals
        all_in_names = list(in_names) + list(out_names)
        if partition_name is not None:
            all_in_names.append(partition_name)

        def _body(*args):
            operands = list(args)
            if partition_name is not None:
                operands.append(partition_id_tensor())
            return tuple(
                _bass_exec_p.bind(
                    *operands,
                    out_avals=tuple(out_avals),
                    in_names=tuple(all_in_names),
                    out_names=tuple(out_names),
                    lowering_input_output_aliases=(),
                    sim_require_finite=True,
                    sim_require_nnan=True,
                    nc=nc,
                )
            )

        devices = jax.devices()[:NCORES]
        mesh = Mesh(np.asarray(devices), ("core",))
        nin = len(in_names) + len(out_names)
        self._fn = jax.jit(
            shard_map(
                _body,
                mesh=mesh,
                in_specs=(PartitionSpec("core"),) * nin,
                out_specs=(PartitionSpec("core"),) * len(out_names),
                check_rep=False,
            ),
            keep_unused=True,
        )
        self._zeros = [
            np.zeros((NCORES * a.shape[0], *a.shape[1:]), a.dtype)
            for a in out_avals
        ]
        self._weight_cache = {}  # name -> (fingerprint, device_array)

    def __call__(self, in_maps):
        concat = {}
        for name in self.in_names:
            arr = np.concatenate([np.asarray(m[name]) for m in in_maps], axis=0)
            if name in self.WEIGHT_NAMES:
                fp = hash(arr.tobytes())
                cached = self._weight_cache.get(name)
                if cached is not None and cached[0] == fp:
                    concat[name] = cached[1]
                else:
                    dev = self.jax.device_put(arr)
                    self._weight_cache[name] = (fp, dev)
                    concat[name] = dev
            else:
                concat[name] = arr
        out_arrs = self._fn(*[concat[n] for n in self.in_names], *self._zeros)
        outs = []
        for c in range(NCORES):
            outs.append({
                name: np.asarray(out_arrs[i]).reshape(
                    NCORES, *self.out_avals[i].shape
                )[c]
                for i, name in enumerate(self.out_names)
            })
        return outs


def get_runner():
    if "runner" not in _CACHE:
        _CACHE["runner"] = _Runner(get_nc())
    return _CACHE["runner"]


def kernel(hidden_states, ln_weight, ln_bias, qkv_weight, qkv_bias,
           proj_weight, proj_bias):
    in_maps = make_in_maps(hidden_states, ln_weight, ln_bias, qkv_weight,
                           qkv_bias, proj_weight, proj_bias)
    outs = get_runner()(in_maps)
    return assemble([o["out"] for o in outs])
